# revision 1
# baseline (speedup 1.0000x reference)
"""DFlashAttention on 8 Trainium2 NeuronCores.

Sharding: data-parallel over batch (4) x tensor-parallel over heads (2).
Core c = 2*b + t handles batch b, q heads [16t, 16t+16), kv heads [4t, 4t+4).
GQA groups (4 q heads per kv head) align with the head split, so attention is
core-local. The output projection is row-sharded over Wo with a pairwise
on-device AllReduce producing the full [128, 4096] output row block per batch.

Per-core pipeline (all matmuls f16, fp32 PSUM accumulation, all attention
matmuls packed to N=512 by processing the 4 q-heads of a GQA group at once):
  Q:   q = rmsnorm(x @ Wq) * rope; rms/rope vectorized over a 4-head group;
       per-head PE transpose into qT4[g] = [d, (4 heads x q)] f16.
  KV:  one DMA per 256-token chunk brings kv_in^T; K and V projected in
       [kv, d] layout (N=512 covers 4 heads); K rmsnorm over the free axis
       (norm weight folded into host-precomputed rope tables), rope
       vectorized over heads, per-head PE transpose; K^T and V spill to DRAM
       in f16.
  ATTN: per 128-kv chunk and group g: S^T = kT_g.T @ qT4[g] (one N=512
       matmul), P = exp(S^T) with no max subtraction (scores are O(1):
       RMS-normed q,k with 1/sqrt(d) folded into the q rope table);
       O^T[g] += V_g.T @ P and l[g] += ones.T @ P accumulate in PSUM over
       8-chunk blocks, then into SBUF. O lands pre-transposed ([d, q]) which
       is exactly the Wo matmul's lhsT layout - no output transposes.
  OUT: attnT = O * (1/l broadcast via ones-outer matmul); partial
       out = attnT.T @ WoT_slice; pairwise AllReduce; write y.
"""
import numpy as np

import concourse.mybir as mybir
import concourse.tile as tile
from concourse import bacc
from concourse.bass_utils import run_bass_kernel_spmd
from concourse.masks import make_identity

B, Q_LEN, CTX, HID = 4, 128, 8192, 4096
H, KVH, D = 32, 8, 128
EPS = 1e-6
N_CORES = 8
TP = 2
HL, GL = H // TP, KVH // TP  # 16 local q heads, 4 local kv heads
AI = HL * D                  # 2048 local attention dims
G4 = GL * D                  # 512 = packed 4-head width

F16 = mybir.dt.float16
F32 = mybir.dt.float32
ALU = mybir.AluOpType
ACTF = mybir.ActivationFunctionType

HC = HID // 128              # 32 hidden chunks
PW = 256                     # kv-projection chunk width
LSC = 1.0 / 8192.0           # l rescale so 1/l stays in f16 normal range


def _attn_blocks(n_chunks):
    sizes = []
    rem = n_chunks
    while rem:
        take = 9 if rem == 9 else min(8, rem)
        sizes.append(take)
        rem -= take
    return sizes


def build(ctx_len=CTX):
    kv_len = ctx_len + Q_LEN
    assert ctx_len % PW == 0
    n_units = ctx_len // PW
    n_chunks = kv_len // 128

    nc = bacc.Bacc("TRN2", target_bir_lowering=False, debug=False,
                   num_devices=N_CORES)

    xT = nc.dram_tensor("xT", [HID, Q_LEN], F16, kind="ExternalInput")
    tT = nc.dram_tensor("tT", [HID, ctx_len], F16, kind="ExternalInput")
    wqT = nc.dram_tensor("wqT", [HID, AI], F16, kind="ExternalInput")
    wkT = nc.dram_tensor("wkT", [HID, G4], F16, kind="ExternalInput")
    wvT = nc.dram_tensor("wvT", [HID, G4], F16, kind="ExternalInput")
    woT = nc.dram_tensor("woT", [HID, AI], F16, kind="ExternalInput")
    cosq4 = nc.dram_tensor("cosq4", [Q_LEN, 512], F16, kind="ExternalInput")
    sinq4 = nc.dram_tensor("sinq4", [Q_LEN, 512], F16, kind="ExternalInput")
    cosk4 = nc.dram_tensor("cosk4", [kv_len, 512], F16, kind="ExternalInput")
    sink4 = nc.dram_tensor("sink4", [kv_len, 512], F16, kind="ExternalInput")
    y = nc.dram_tensor("y", [Q_LEN, AI], F32, kind="ExternalOutput")

    eps_ap = [None]

    def rope_block(pool, src_ap, cos_ap, sin_ap, tag):
        """src [128, 512] f32 (PSUM) -> rms-normed + roped f16 [128, 512].
        cos/sin are f16 [128, 512] with norm weights pre-folded."""
        scr = pool.tile([128, 128], F32, tag=f"{tag}scr", name="scr")
        ms4 = pool.tile([128, 4], F32, tag=f"{tag}ms4", name="ms4")
        for g in range(4):
            nc.scalar.activation(scr[:], src_ap[:, 128 * g:128 * (g + 1)],
                                 func=ACTF.Square,
                                 accum_out=ms4[:, g:g + 1])
        rms4 = pool.tile([128, 4], F32, tag=f"{tag}rms4", name="rms4")
        nc.scalar.activation(rms4[:], ms4[:], func=ACTF.Sqrt,
                             scale=1.0 / D, bias=eps_ap[0])
        inv4 = pool.tile([128, 4], F32, tag=f"{tag}inv4", name="inv4")
        nc.vector.reciprocal(inv4[:], rms4[:])
        kn = pool.tile([128, 512], F16, tag=f"{tag}kn", name="kn")
        for g in range(4):
            nc.scalar.activation(kn[:, 128 * g:128 * (g + 1)],
                                 src_ap[:, 128 * g:128 * (g + 1)],
                                 func=ACTF.Copy, scale=inv4[:, g:g + 1])
        knv = kn[:].rearrange("p (g d) -> p g d", d=D)
        sinv = sin_ap.rearrange("p (g d) -> p g d", d=D)
        t1 = pool.tile([128, 512], F16, tag=f"{tag}t1", name="t1")
        nc.vector.tensor_mul(t1[:], kn[:], cos_ap)
        t2 = pool.tile([128, 512], F16, tag=f"{tag}t2", name="t2")
        t2v = t2[:].rearrange("p (g d) -> p g d", d=D)
        nc.vector.scalar_tensor_tensor(
            t2v[:, :, 0:64], knv[:, :, 64:128], -1.0, sinv[:, :, 0:64],
            op0=ALU.mult, op1=ALU.mult)
        nc.vector.tensor_mul(t2v[:, :, 64:128], knv[:, :, 0:64],
                             sinv[:, :, 64:128])
        kf = pool.tile([128, 512], F16, tag=f"{tag}kf", name="kf")
        nc.vector.tensor_add(kf[:], t1[:], t2[:])
        return kf

    with tile.TileContext(nc) as tc:
        with (
            tc.tile_pool(name="dram", bufs=1, space="DRAM") as dpool,
            tc.tile_pool(name="const", bufs=1) as cpool,
            tc.tile_pool(name="qt", bufs=1) as qtpool,
            tc.tile_pool(name="oacc", bufs=1) as opool,
        ):
            kt_store = dpool.tile([G4, kv_len], F16)
            v_store = dpool.tile([kv_len, G4], F16)
            ag_in = dpool.tile([G4, 512], F16)
            ag_out = dpool.tile([TP * G4, 512], F16)

            cq_t = cpool.tile([Q_LEN, 512], F16)
            sq_t = cpool.tile([Q_LEN, 512], F16)
            ident = cpool.tile([128, 128], F16)
            ones = cpool.tile([128, 128], F16)
            eps_t = cpool.tile([128, 1], F32)

            qT4 = [qtpool.tile([D, 512], F16, name=f"qT4_{g}")
                   for g in range(GL)]
            o_sb = [opool.tile([D, 512], F32, name=f"osb{g}")
                    for g in range(GL)]
            l_sb = [opool.tile([1, 512], F32, name=f"lsb{g}")
                    for g in range(GL)]

            with (
                tc.tile_pool(name="xw", bufs=1) as xwpool,
                tc.tile_pool(name="rp", bufs=2) as rp,
            ):
                xtu = xwpool.tile([128, HC * Q_LEN], F16, name="xtu")
                nc.sync.dma_start(
                    xtu[:].rearrange("p (c q) -> p c q", q=Q_LEN),
                    xT[:, :].rearrange("(c p) q -> p c q", p=128))
                nc.sync.dma_start(cq_t[:], cosq4[:, :])
                nc.sync.dma_start(sq_t[:], sinq4[:, :])
                make_identity(nc, ident[:])
                nc.vector.memset(ones[:], 1.0)
                nc.vector.memset(eps_t[:], EPS)
                eps_ap[0] = eps_t[:]
                wku = xwpool.tile([128, HC * G4], F16, name="wku")
                wvu = xwpool.tile([128, HC * G4], F16, name="wvu")
                xv = xtu[:].rearrange("p (c q) -> p c q", q=Q_LEN)
                wkv_ = wku[:].rearrange("p (c w) -> p c w", w=G4)
                wvv = wvu[:].rearrange("p (c w) -> p c w", w=G4)

                # ================= Q phase =================
                with (
                    tc.tile_pool(name="qw", bufs=2) as qw,
                    tc.tile_pool(name="qpsum", bufs=2, space="PSUM") as qpp,
                    tc.tile_pool(name="qtps", bufs=2, space="PSUM") as qtp,
                ):
                    for g in range(GL):
                        wqu = qw.tile([128, HC * 512], F16, tag="wqu",
                                      name="wqu")
                        nc.sync.dma_start(
                            wqu[:].rearrange("p (c w) -> p c w", w=512),
                            wqT[:, 512 * g:512 * (g + 1)].rearrange(
                                "(c p) w -> p c w", p=128))
                        wqv = wqu[:].rearrange("p (c w) -> p c w", w=512)
                        qps = qpp.tile([Q_LEN, 512], F32, tag="qps",
                                       name="qps")
                        for hc in range(HC):
                            nc.tensor.matmul(qps[:], xv[:, hc, :],
                                             wqv[:, hc, :],
                                             start=(hc == 0),
                                             stop=(hc == HC - 1))
                        qf = rope_block(rp, qps[:], cq_t[:], sq_t[:], "q")
                        qtps = qtp.tile([D, 512], F16, tag="qtps",
                                        name="qtps")
                        for hh in range(4):
                            nc.tensor.transpose(
                                qtps[:, 128 * hh:128 * (hh + 1)],
                                qf[:, 128 * hh:128 * (hh + 1)], ident[:])
                        nc.scalar.copy(qT4[g][:], qtps[:])

                # wk/wv loads, split so the first KV matmuls start early
                for piece in range(4):
                    hc0, hc1 = 8 * piece, 8 * (piece + 1)
                    nc.sync.dma_start(
                        wku[:].rearrange("p (c w) -> p c w", w=G4)[:, hc0:hc1, :],
                        wkT[128 * hc0:128 * hc1, :].rearrange(
                            "(c p) w -> p c w", p=128))
                    nc.sync.dma_start(
                        wvu[:].rearrange("p (c w) -> p c w", w=G4)[:, hc0:hc1, :],
                        wvT[128 * hc0:128 * hc1, :].rearrange(
                            "(c p) w -> p c w", p=128))

                # ================= KV projection =================
                with (
                    tc.tile_pool(name="tt", bufs=2) as ttp,
                    tc.tile_pool(name="cs", bufs=3) as csp,
                    tc.tile_pool(name="kv_sb", bufs=3) as kvs,
                    tc.tile_pool(name="kpsum", bufs=3, space="PSUM") as kpp,
                    tc.tile_pool(name="vpsum", bufs=2, space="PSUM") as vpp,
                    tc.tile_pool(name="ktps", bufs=2, space="PSUM") as ktpp,
                ):
                    for u in range(n_units + 1):
                        final = u == n_units
                        w = 128 if final else PW
                        ns = w // 128
                        pos0 = ctx_len if final else PW * u
                        if final:
                            ttv = xv
                        else:
                            ttu = ttp.tile([128, HC * PW], F16, tag="tt",
                                           name="ttu")
                            nc.sync.dma_start(
                                ttu[:].rearrange("p (c w) -> p c w", w=PW),
                                tT[:, pos0:pos0 + w].rearrange(
                                    "(c p) w -> p c w", p=128))
                            ttv = ttu[:].rearrange("p (c w) -> p c w", w=PW)
                        ck = csp.tile([128, ns * 512], F16, tag="ck",
                                      name="ck")
                        nc.sync.dma_start(
                            ck[:].rearrange("p (s w) -> p s w", w=512),
                            cosk4[pos0:pos0 + w, :].rearrange(
                                "(s p) w -> p s w", p=128))
                        sk = csp.tile([128, ns * 512], F16, tag="sk",
                                      name="sk")
                        nc.sync.dma_start(
                            sk[:].rearrange("p (s w) -> p s w", w=512),
                            sink4[pos0:pos0 + w, :].rearrange(
                                "(s p) w -> p s w", p=128))

                        for s in range(ns):
                            kps = kpp.tile([128, G4], F32, tag="kps",
                                           name="kps")
                            vps = vpp.tile([128, G4], F32, tag="vps",
                                           name="vps")
                            for hc in range(HC):
                                tts = ttv[:, hc, 128 * s:128 * (s + 1)]
                                nc.tensor.matmul(kps[:], tts,
                                                 wkv_[:, hc, :],
                                                 start=(hc == 0),
                                                 stop=(hc == HC - 1))
                                nc.tensor.matmul(vps[:], tts,
                                                 wvv[:, hc, :],
                                                 start=(hc == 0),
                                                 stop=(hc == HC - 1))
                            kf = rope_block(
                                rp, kps[:],
                                ck[:, 512 * s:512 * (s + 1)],
                                sk[:, 512 * s:512 * (s + 1)], "k")
                            ktps = ktpp.tile([D, 512], F16, tag="ktps",
                                             name="ktps")
                            for g in range(4):
                                nc.tensor.transpose(
                                    ktps[:, 128 * g:128 * (g + 1)],
                                    kf[:, 128 * g:128 * (g + 1)], ident[:])
                            kt4 = kvs.tile([D, 512], F16, tag="kt4",
                                           name="kt4")
                            nc.scalar.copy(kt4[:], ktps[:])
                            pos = pos0 + 128 * s
                            nc.sync.dma_start(
                                kt_store[:, pos:pos + 128].rearrange(
                                    "(g d) k -> d g k", g=4),
                                kt4[:].rearrange("p (g k) -> p g k", k=128))
                            vsb = kvs.tile([128, G4], F16, tag="vsb",
                                           name="vsb")
                            nc.scalar.copy(vsb[:], vps[:])
                            nc.sync.dma_start(
                                v_store[pos:pos + 128, :], vsb[:])

            # ================= attention =================
            with (
                tc.tile_pool(name="ktb", bufs=4) as ktb,
                tc.tile_pool(name="vb", bufs=11) as vbp,
                tc.tile_pool(name="pt", bufs=38) as ptp,
                tc.tile_pool(name="wo", bufs=20) as wop,
                tc.tile_pool(name="osb", bufs=2) as osb,
                tc.tile_pool(name="at", bufs=1) as atp,
                tc.tile_pool(name="ag", bufs=1) as agp,
            ):
                NJ = AI // 512
                NH = TP * HL
                wo_tiles = {}

                def load_wou(h):
                    wou = wop.tile([128, AI], F16, tag="wou", name="wou")
                    nc.sync.dma_start(
                        wou[:], woT[128 * h:128 * (h + 1), :])
                    wo_tiles[h] = wou

                c0 = 0
                blocks = _attn_blocks(n_chunks)
                wo_at = 1 if len(blocks) > 1 else 0
                attn_psums = (
                    tc.tile_pool(name="stp", bufs=3, space="PSUM"),
                    tc.tile_pool(name="op", bufs=3, space="PSUM"),
                    tc.tile_pool(name="lp", bufs=2, space="PSUM"),
                )
                stp = attn_psums[0].__enter__()
                opp = attn_psums[1].__enter__()
                lpp = attn_psums[2].__enter__()
                wo_next = [0]

                def prefetch_wou(n):
                    while wo_next[0] < min(20, NH) and n > 0:
                        load_wou(wo_next[0])
                        wo_next[0] += 1
                        n -= 1

                for blk, nb in enumerate(blocks):
                    if blk >= wo_at:
                        prefetch_wou(3)
                    pt_tiles = []
                    v_tiles = []
                    for ci in range(nb):
                        c = c0 + ci
                        kt4 = ktb.tile([D, G4], F16, tag="kt", name="kt4a")
                        nc.sync.dma_start(
                            kt4[:].rearrange("p (g k) -> p g k", k=128),
                            kt_store[:, 128 * c:128 * (c + 1)].rearrange(
                                "(g d) k -> d g k", g=4))
                        v_t = vbp.tile([128, G4], F16, tag="vt", name="v_t")
                        nc.sync.dma_start(
                            v_t[:], v_store[128 * c:128 * (c + 1), :])
                        v_tiles.append(v_t)
                        pts = []
                        for g in range(GL):
                            sp = stp.tile([128, 512], F32, tag="stps",
                                          name="sp")
                            nc.tensor.matmul(
                                sp[:], kt4[:, 128 * g:128 * (g + 1)],
                                qT4[g][:], start=True, stop=True)
                            pt = ptp.tile([128, 512], F16, tag="pt",
                                          name="pt")
                            nc.scalar.activation(pt[:], sp[:], func=ACTF.Exp)
                            pts.append(pt)
                        pt_tiles.append(pts)
                    for g in range(GL):
                        ops = opp.tile([D, 512], F32, tag="op", name="ops")
                        lps = lpp.tile([1, 512], F32, tag="lp", name="lps")
                        for ci in range(nb):
                            nc.tensor.matmul(
                                ops[:], v_tiles[ci][:, 128 * g:128 * (g + 1)],
                                pt_tiles[ci][g][:],
                                start=(ci == 0), stop=(ci == nb - 1))
                            nc.tensor.matmul(
                                lps[:], ones[:, 0:1], pt_tiles[ci][g][:],
                                start=(ci == 0), stop=(ci == nb - 1))
                        if blk == 0:
                            nc.vector.tensor_copy(o_sb[g][:], ops[:])
                            nc.vector.tensor_copy(l_sb[g][:], lps[:])
                        else:
                            nc.vector.tensor_add(o_sb[g][:], o_sb[g][:],
                                                 ops[:])
                            nc.vector.tensor_add(l_sb[g][:], l_sb[g][:],
                                                 lps[:])
                    c0 += nb
                for p in reversed(attn_psums):
                    p.__exit__(None, None, None)

                # ============ output projection (allgather + full Wo) ======
                with (
                    tc.tile_pool(name="wops", bufs=4, space="PSUM") as wopp,
                    tc.tile_pool(name="bcps", bufs=2, space="PSUM") as bcpp,
                ):
                    aT4 = [atp.tile([D, 512], F16, name=f"aT4_{g}")
                           for g in range(GL)]
                    for g in range(GL):
                        lsc = osb.tile([1, 512], F32, tag="lsc", name="lsc")
                        nc.vector.tensor_scalar_mul(lsc[:], l_sb[g][:], LSC)
                        invl = osb.tile([1, 512], F16, tag="invl",
                                        name="invl")
                        with nc.allow_low_precision("1/l in f16"):
                            nc.vector.reciprocal(invl[:], lsc[:])
                        for hh in range(4):
                            cs = slice(128 * hh, 128 * (hh + 1))
                            bcp = bcpp.tile([128, 128], F32, tag="bcp",
                                            name="bcp")
                            nc.tensor.matmul(bcp[:], ones[0:1, :],
                                             invl[:, cs])
                            nc.vector.scalar_tensor_tensor(
                                aT4[g][:, cs], o_sb[g][:, cs], LSC, bcp[:],
                                op0=ALU.mult, op1=ALU.mult)
                        nc.sync.dma_start(
                            ag_in[128 * g:128 * (g + 1), :], aT4[g][:])
                    nc.gpsimd.collective_compute(
                        "AllGather",
                        ALU.bypass,
                        replica_groups=[[0, 1], [2, 3], [4, 5], [6, 7]],
                        ins=[ag_in.opt()],
                        outs=[ag_out.opt()],
                    )
                    ag_sb = []
                    for t in range(TP):
                        a_sb = agp.tile([128, GL * 512], F16,
                                        name=f"ag_sb{t}")
                        nc.sync.dma_start(
                            a_sb[:].rearrange("p (g w) -> p g w", w=512),
                            ag_out[G4 * t:G4 * (t + 1), :].rearrange(
                                "(g p) w -> p g w", p=128))
                        ag_sb.append(a_sb)
                    out_sb = agp.tile([Q_LEN, AI], F32, name="out_sb")
                    prefetch_wou(20)
                    wps = [wopp.tile([Q_LEN, 512], F32, tag="wops",
                                     name="wps") for _ in range(NJ)]
                    for h in range(NH):
                        if h + 20 < NH:
                            load_wou(h + 20)
                        t, g, hh = h // 16, (h % 16) // 4, h % 4
                        lhs = ag_sb[t][:].rearrange("p (g w) -> p g w", w=512)
                        for j in range(NJ):
                            nc.tensor.matmul(
                                wps[j][:],
                                lhs[:, g, 128 * hh:128 * (hh + 1)],
                                wo_tiles[h][:, 512 * j:512 * (j + 1)],
                                start=(h == 0), stop=(h == NH - 1))
                    for j in range(NJ):
                        nc.vector.tensor_copy(
                            out_sb[:, 512 * j:512 * (j + 1)], wps[j][:])
                        nc.sync.dma_start(y[:, 512 * j:512 * (j + 1)],
                                          out_sb[:, 512 * j:512 * (j + 1)])

    nc.compile()
    return nc


def host_prep(hidden_states, target_hidden, cos, sin, Wq, Wk, Wv, Wo,
              q_norm_w, k_norm_w, ctx_len=CTX):
    """Build the 8 per-core input maps from full inputs (numpy, host side)."""
    kv_len = ctx_len + Q_LEN
    f16 = np.float16
    f32 = np.float32

    qw = np.asarray(q_norm_w, f32)
    kw = np.asarray(k_norm_w, f32)
    qw_rot = np.concatenate([qw[64:], qw[:64]])
    kw_rot = np.concatenate([kw[64:], kw[:64]])
    scale = np.float32(D ** -0.5)

    per_b = {}
    for b in range(B):
        cq = np.asarray(cos[b, ctx_len:kv_len], f32)
        sq = np.asarray(sin[b, ctx_len:kv_len], f32)
        ckv = np.asarray(cos[b, :kv_len], f32)
        skv = np.asarray(sin[b, :kv_len], f32)
        per_b[b] = {
            "xT": np.ascontiguousarray(hidden_states[b].T).astype(f16),
            "tT": np.ascontiguousarray(target_hidden[b].T).astype(f16),
            "cosq4": np.ascontiguousarray(
                np.tile((cq * qw * scale).astype(f16), (1, 4))),
            "sinq4": np.ascontiguousarray(
                np.tile((sq * qw_rot * scale).astype(f16), (1, 4))),
            "cosk4": np.ascontiguousarray(
                np.tile((ckv * kw).astype(f16), (1, 4))),
            "sink4": np.ascontiguousarray(
                np.tile((skv * kw_rot).astype(f16), (1, 4))),
        }
    per_t = {}
    for t in range(TP):
        per_t[t] = {
            "wqT": np.ascontiguousarray(
                Wq[AI * t:AI * (t + 1), :].T).astype(f16),
            "wkT": np.ascontiguousarray(
                Wk[G4 * t:G4 * (t + 1), :].T).astype(f16),
            "wvT": np.ascontiguousarray(
                Wv[G4 * t:G4 * (t + 1), :].T).astype(f16),
            "woT": np.ascontiguousarray(
                Wo[AI * t:AI * (t + 1), :].T).astype(f16),
        }
    in_maps = []
    for c in range(N_CORES):
        b, t = c // TP, c % TP
        m = {}
        m.update(per_b[b])
        m.update(per_t[t])
        in_maps.append(m)
    return in_maps


_CACHE = {}


def _get_module(ctx_len=CTX):
    if ctx_len not in _CACHE:
        _CACHE[ctx_len] = build(ctx_len)
    return _CACHE[ctx_len]


def kernel(hidden_states, target_hidden, cos, sin, Wq, Wk, Wv, Wo,
           q_norm_w, k_norm_w):
    args = [np.asarray(a) for a in (hidden_states, target_hidden, cos, sin,
                                    Wq, Wk, Wv, Wo, q_norm_w, k_norm_w)]
    nc = _get_module(CTX)
    in_maps = host_prep(*args, ctx_len=CTX)
    res = run_bass_kernel_spmd(nc, in_maps, core_ids=list(range(N_CORES)))
    out = np.stack(
        [np.concatenate([res.results[TP * b + t]["y"] for t in range(TP)],
                        axis=1) for b in range(B)], axis=0)
    return out.astype(np.float32)

